# revision 20
# baseline (speedup 1.0000x reference)
"""Trainium2 Bass kernel for the CudaTensorProduct problem.

out[b, o] = sum_k palette[cb_idx[k]] * in1[b, i1[k]] * in2[b, i2[k]]
with the COO structure of an e3nn-style full tensor product
(irreps '32x0e+16x1o+8x2e' x itself, all output paths).

Strategy (8 NeuronCores, SPMD):
  - Data-parallel batch shard: 2048 rows -> 8 cores x 256 rows.
  - Per core: 2 partition-tiles of 128 batches, fused into single DVE
    instructions via an extra access-pattern dim.
  - Per (l1,l2) block: one scalar_tensor_tensor outer product builds the
    full product table T[(i,m1),(j,m2)] = x1[i,m1]*x2[j,m2]; each output
    column (l3,m3) is then one direct STT (first Clebsch-Gordan term) plus
    in-place STT accumulates (remaining terms) reading T with strided APs.
  - Output [128, 2, 14400] stays resident in SBUF; DMA'd out per
    (block,l3) column range so stores overlap compute.
"""

import math

import numpy as np

# (mul, l, parity) for irreps_in1 == irreps_in2 == '32x0e+16x1o+8x2e'
IRREPS = [(32, 0, 1), (16, 1, -1), (8, 2, 1)]
BATCH = 2048
DIM = 120  # sum(mul * (2l+1))
N_CORES = 8
ROWS_PER_CORE = BATCH // N_CORES  # 256
P = 128  # partitions
TILES = ROWS_PER_CORE // P  # 2


# ---------------------------------------------------------------------------
# Wigner 3j structure (mirrors reference.py exactly)
# ---------------------------------------------------------------------------

def _fact(n):
    return math.factorial(n)


def _w3j_entry(j1, j2, j3, m1, m2, m3):
    if m1 + m2 + m3 != 0:
        return 0.0
    t_min = max(0, j2 - j3 - m1, j1 - j3 + m2)
    t_max = min(j1 + j2 - j3, j1 - m1, j2 + m2)
    if t_max < t_min:
        return 0.0
    s = 0.0
    for t in range(t_min, t_max + 1):
        s += (-1) ** t / (
            _fact(t) * _fact(j3 - j2 + t + m1) * _fact(j3 - j1 + t - m2)
            * _fact(j1 + j2 - j3 - t) * _fact(j1 - t - m1) * _fact(j2 - t + m2))
    delta = math.sqrt(_fact(j1 + j2 - j3) * _fact(j1 - j2 + j3)
                      * _fact(-j1 + j2 + j3) / _fact(j1 + j2 + j3 + 1))
    pref = math.sqrt(_fact(j1 + m1) * _fact(j1 - m1) * _fact(j2 + m2)
                     * _fact(j2 - m2) * _fact(j3 + m3) * _fact(j3 - m3))
    return (-1) ** (j1 - j2 - m3) * delta * pref * s


def _w3j_complex(j1, j2, j3):
    W = np.zeros((2 * j1 + 1, 2 * j2 + 1, 2 * j3 + 1))
    for m1 in range(-j1, j1 + 1):
        for m2 in range(-j2, j2 + 1):
            m3 = -(m1 + m2)
            if abs(m3) <= j3:
                W[m1 + j1, m2 + j2, m3 + j3] = _w3j_entry(j1, j2, j3, m1, m2, m3)
    return W


def _c2r(l):
    U = np.zeros((2 * l + 1, 2 * l + 1), dtype=np.complex128)
    U[l, l] = 1.0
    rs2 = 1.0 / math.sqrt(2.0)
    for m in range(1, l + 1):
        U[l + m, l + m] = (-1) ** m * rs2
        U[l + m, l - m] = rs2
        U[l - m, l - m] = 1j * rs2
        U[l - m, l + m] = -1j * (-1) ** m * rs2
    return U


def _w3j_real(l1, l2, l3):
    W = _w3j_complex(l1, l2, l3).astype(np.complex128)
    C = np.einsum('abc,ia,jb,kc->ijk', W, _c2r(l1), _c2r(l2), _c2r(l3))
    if np.linalg.norm(C.imag) > np.linalg.norm(C.real):
        C = np.ascontiguousarray(C.imag)
    else:
        C = np.ascontiguousarray(C.real)
    C[np.abs(C) < 1e-12] = 0.0
    return C


# ---------------------------------------------------------------------------
# Compute plan: per (l1,l2) block, per l3: column base + term list per m3
# ---------------------------------------------------------------------------

class BlockPlan:
    def __init__(self, l1, l2, mul1, mul2, o1, o2):
        self.l1, self.l2 = l1, l2
        self.mul1, self.mul2 = mul1, mul2
        self.d1, self.d2 = 2 * l1 + 1, 2 * l2 + 1
        self.o1, self.o2 = o1, o2
        # per l3: (col_base, d3, terms) with terms[m3] = [(m1, m2, val), ...]
        self.l3s = []


def build_plan():
    """Reconstruct the exact output layout of reference._build_structure."""
    # 1. path bookkeeping, same iteration order as the reference
    entries_by_l3 = {}  # (l3,p3) -> list of (l1, l2, o1, o2, mul1, mul2)
    off1 = 0
    blocks = []
    for mul1, l1, p1 in IRREPS:
        off2 = 0
        for mul2, l2, p2 in IRREPS:
            blk = BlockPlan(l1, l2, mul1, mul2, off1, off2)
            blocks.append(blk)
            for l3 in range(abs(l1 - l2), l1 + l2 + 1):
                p3 = (p1 * p2 + 1) // 2
                entries_by_l3.setdefault((l3, p3), []).append(blk)
            off2 += (2 * l2 + 1) * mul2
        off1 += (2 * l1 + 1) * mul1

    # 2. group order + column bases
    l3s = sorted(entries_by_l3.keys(), key=lambda x: 2 * (x[0] + 1) + x[1])
    row = 0
    for (l3, p3) in l3s:
        for blk in entries_by_l3[(l3, p3)]:
            d3 = 2 * l3 + 1
            C = _w3j_real(blk.l1, blk.l2, l3) * math.sqrt(d3)
            terms = []
            for m3 in range(d3):
                tl = []
                for m2 in range(blk.d2):
                    for m1 in range(blk.d1):
                        v = C[m1, m2, m3]
                        if v != 0.0:
                            tl.append((m1, m2, float(v)))
                terms.append(tl)
            blk.l3s.append((row, d3, terms))
            row += blk.mul1 * blk.mul2 * d3
    assert row == 14400
    return blocks


def plan_coo(blocks):
    """Reproduce the reference COO arrays from the plan (for verification)."""
    i1l, i2l, iol, vl = [], [], [], []
    # reference emits: for each (l3,p3) group (sorted), for each path
    # (block-order, then i, then j), for m3, m2, m1
    ordered = []  # (group_key, block_seq, blk, l3_entry)
    for bi, blk in enumerate(blocks):
        for (cb, d3, terms) in blk.l3s:
            ordered.append((cb, blk, d3, terms))
    ordered.sort(key=lambda x: x[0])
    for cb, blk, d3, terms in ordered:
        for i in range(blk.mul1):
            for j in range(blk.mul2):
                col0 = cb + (i * blk.mul2 + j) * d3
                for m3 in range(d3):
                    for (m1, m2, v) in terms[m3]:
                        i1l.append(blk.o1 + i * blk.d1 + m1)
                        i2l.append(blk.o2 + j * blk.d2 + m2)
                        iol.append(col0 + m3)
                        vl.append(v)
    return (np.asarray(i1l, np.int32), np.asarray(i2l, np.int32),
            np.asarray(iol, np.int32), np.asarray(vl, np.float64))


# ---------------------------------------------------------------------------
# Bass kernel construction
# ---------------------------------------------------------------------------

_BASS_CACHE = {}

# engine offloads: single-term blocks on GPSIMD, first CG terms on ScalarE
OFFLOAD_S1 = True
OFFLOAD_FIRSTS = True


def _insert_bcast(ap_obj, axis, count):
    """Insert a step-0 (broadcast) dim at free-dim position `axis` (0-based
    counting after the partition dim)."""
    from concourse.bass import AP
    dims = list(ap_obj.ap)
    dims.insert(axis + 1, [0, count])
    return AP(tensor=ap_obj.tensor, offset=ap_obj.offset, ap=dims)


def _sub_ap(base_ap, extra_offset, dims):
    """AP at base + extra_offset (element units) with custom free dims.
    Keeps the (possibly symbolic) tile offset by slicing first."""
    from concourse.bass import AP
    s = base_ap[:, extra_offset:extra_offset + 1]
    return AP(tensor=s.tensor, offset=s.offset, ap=[list(s.ap[0])] + dims)


def build_bass():
    import concourse.bass as bass
    import concourse.mybir as mybir
    from concourse.tile import TileContext

    f32 = mybir.dt.float32
    MUL = mybir.AluOpType.mult
    ADD = mybir.AluOpType.add

    blocks = build_plan()

    nc = bass.Bass("TRN2", target_bir_lowering=False)
    in1_d = nc.dram_tensor("in1", [ROWS_PER_CORE, DIM], f32, kind="ExternalInput")
    in2_d = nc.dram_tensor("in2", [ROWS_PER_CORE, DIM], f32, kind="ExternalInput")
    out_d = nc.dram_tensor("out", [ROWS_PER_CORE, 14400], f32, kind="ExternalOutput")

    # DRAM views with rows split as (tile, partition)
    in1_v = in1_d[:].rearrange("(t p) c -> p t c", t=TILES)
    in2_v = in2_d[:].rearrange("(t p) c -> p t c", t=TILES)
    out_v = out_d[:].rearrange("(t p) c -> p t c", t=TILES)

    # s=1 blocks first (cheap, cover many early columns), then multi-term
    # blocks ordered so early output chunks complete sooner
    def block_order(blk):
        s1 = blk.l1 == 0 or blk.l2 == 0
        rank = {(1, 1): 0, (2, 2): 1, (1, 2): 2, (2, 1): 3}
        return (0, 0) if s1 else (1, rank[(blk.l1, blk.l2)])
    ordered_blocks = sorted(blocks, key=block_order)

    # Hardware constraint: TensorScalarPtr (STT / tensor_scalar) APs are
    # limited to partition + 2 free dims. The SBUF output buffer therefore
    # uses a block-major region layout [t][i][j][m3] per (block, l3) so the
    # tile dim t combines with i into a single AP dim (t stride == mul1 * i
    # stride), keeping every instruction 3D while still fusing both tiles.

    # Output path: compute lands in block-major SBUF regions; the (otherwise
    # idle) ACT engine reassembles regions into column-ordered chunk buffers
    # which are stored with exactly NCHUNK large DMAs. Total DMA count must
    # stay <= 8: each extra DMA would reuse a completion-semaphore lane and
    # need a second sync-wait, which the DMA ISA encoding cannot carry.
    NCHUNK = 6
    CW = 14400 // NCHUNK  # chunk width in output columns

    # map each (block, l3) to its region offset in ob (assigned in emission
    # order below) and remember (cb, w, reg) for the copy stage
    with TileContext(nc) as tc:
        with tc.tile_pool(name="io", bufs=1) as iop, \
             tc.tile_pool(name="tab", bufs=1) as tabp, \
             tc.tile_pool(name="chk", bufs=2) as chkp:
            x1 = iop.tile([P, TILES, DIM], f32)
            x2 = iop.tile([P, TILES, DIM], f32)
            x1n = iop.tile([P, TILES, DIM], f32)  # -x1, for c=-1 on GPSIMD
            ob = iop.tile([P, TILES * 14400], f32)
            warm = iop.tile([P, 4], f32)
            nc.sync.dma_start(out=x1[:], in_=in1_v)
            nc.sync.dma_start(out=x2[:], in_=in2_v)
            # The compute ISA encodings have a single sync-wait slot, but an
            # instruction touching both inputs would need waits on two DMA
            # semaphore lanes. Absorb each DMA wait into each engine's
            # clock with trivial ops so no instruction carries >1 wait.
            nc.vector.tensor_copy(out=warm[:, 0:1], in_=x1[:, 0, 0:1])
            nc.vector.tensor_copy(out=warm[:, 1:2], in_=x2[:, 0, 0:1])
            if OFFLOAD_S1:
                # GPSIMD warm-ups: the x1 negate doubles as the lane-0 absorb
                nc.gpsimd.tensor_scalar_mul(x1n[:], x1[:], -1.0)
                nc.gpsimd.tensor_scalar_mul(warm[:, 2:3], x2[:, 0, 0:1], 1.0)

            regions = []  # (cb, w, reg_off)
            reg = 0  # region cursor into ob (per-partition elements)
            for blk in ordered_blocks:
                mul1, mul2, d1, d2 = blk.mul1, blk.mul2, blk.d1, blk.d2
                m1w, m2w = mul1 * d1, mul2 * d2
                s1 = blk.l1 == 0 or blk.l2 == 0

                if not s1:
                    # product table T[t][(i,m1)][(j,m2)], per-tile products
                    sz = m1w * m2w
                    T = tabp.tile([P, TILES * sz], f32, tag="T")
                    for t in range(TILES):
                        t_ap = T[:, t * sz:(t + 1) * sz].rearrange(
                            "p (a b) -> p a b", a=m1w)
                        a_ap = _insert_bcast(
                            x1[:, t, blk.o1:blk.o1 + m1w], 1, m2w)
                        b_ap = _insert_bcast(
                            x2[:, t, blk.o2:blk.o2 + m2w], 0, m1w)
                        nc.vector.tensor_mul(out=t_ap, in0=a_ap, in1=b_ap)

                    def t_read(m1, m2):
                        # [p, (t i), j] over T at fixed (m1, m2)
                        return _sub_ap(T[:], m1 * m2w + m2,
                                       [[d1 * m2w, TILES * mul1], [d2, mul2]])

                for (cb, d3, terms) in blk.l3s:
                    w = mul1 * mul2 * d3

                    if s1:
                        # single-term block, c = +-1: GPSIMD tensor_mul with
                        # x1 (c=1) or the negated copy x1n (c=-1)
                        c = float(np.float32(terms[0][0][2]))
                        assert c in (1.0, -1.0), c
                        # all m3 share c in fp32 (diagonal CG, equal by
                        # rotational invariance; float64 noise only)
                        assert all(float(np.float32(v)) == c
                                   for tl in terms for (_, _, v) in tl)
                        xs = x1 if c > 0 else x1n
                        for t in range(TILES):
                            base = ob[:, reg + t * w: reg + (t + 1) * w]
                            if blk.l1 == 0:
                                # out[i][(j m3)] = c * x1[i] * x2[(j m3)]
                                o_ap = base.rearrange(
                                    "p (i jm) -> p i jm", i=mul1)
                                a_ap = _insert_bcast(
                                    xs[:, t, blk.o1:blk.o1 + mul1], 1, m2w)
                                b_ap = _insert_bcast(
                                    x2[:, t, blk.o2:blk.o2 + m2w], 0, mul1)
                                if OFFLOAD_S1:
                                    nc.gpsimd.tensor_mul(
                                        out=o_ap, in0=a_ap, in1=b_ap)
                                else:
                                    nc.vector.scalar_tensor_tensor(
                                        o_ap, a_ap, c, b_ap, MUL, MUL)
                            else:
                                # l2 == 0: out[i][j][m3] = c * x1[i,m3] * x2[j]
                                for m3 in range(d3):
                                    o_ap = _sub_ap(
                                        ob[:], reg + t * w + m3,
                                        [[mul2 * d3, mul1], [d3, mul2]])
                                    a_ap = xs[:, t, blk.o1:blk.o1 + m1w].rearrange(
                                        "p (q a) -> p q a", a=d1)[:, :, m3]
                                    a_ap = _insert_bcast(a_ap, 1, mul2)
                                    b_ap = _insert_bcast(
                                        x2[:, t, blk.o2:blk.o2 + mul2], 0, mul1)
                                    if OFFLOAD_S1:
                                        nc.gpsimd.tensor_mul(
                                            out=o_ap, in0=a_ap, in1=b_ap)
                                    else:
                                        nc.vector.scalar_tensor_tensor(
                                            o_ap, a_ap, c, b_ap, MUL, MUL)
                    else:
                        # block-major col AP: [p, (t i), j], offset m3
                        for m3 in range(d3):
                            o_ap = _sub_ap(
                                ob[:], reg + m3,
                                [[mul2 * d3, TILES * mul1], [d3, mul2]])
                            first = True
                            for (m1, m2, v) in terms[m3]:
                                tt = t_read(m1, m2)
                                if first:
                                    if OFFLOAD_FIRSTS:
                                        nc.scalar.mul(o_ap, tt, v)
                                    else:
                                        nc.vector.tensor_scalar_mul(
                                            o_ap, tt, v)
                                    first = False
                                else:
                                    # in-place: out = (T * v) + out
                                    nc.vector.scalar_tensor_tensor(
                                        o_ap, tt, v, o_ap, MUL, ADD)

                    regions.append((cb, w, reg))
                    reg += TILES * w
            assert reg == TILES * 14400

            # copy stage (ACT) + chunked stores: walk chunks in column
            # order; for each, copy every intersecting region piece from
            # its block-major home into the chunk's [t][col] layout, then
            # store the chunk with one large DMA.
            regions.sort()
            for ci in range(NCHUNK):
                c0, c1 = ci * CW, (ci + 1) * CW
                chunk = chkp.tile([P, TILES * CW], f32, tag="chunk")
                for (cb, w, roff) in regions:
                    lo, hi = max(cb, c0), min(cb + w, c1)
                    if lo >= hi:
                        continue
                    for t in range(TILES):
                        src = ob[:, roff + t * w + (lo - cb):
                                 roff + t * w + (hi - cb)]
                        dst = chunk[:, t * CW + (lo - c0): t * CW + (hi - c0)]
                        nc.scalar.copy(out=dst, in_=src)
                nc.sync.dma_start(
                    out=out_v[:, :, c0:c1],
                    in_=chunk[:].rearrange("p (t w) -> p t w", t=TILES))
                # Absorb this DMA's completion into the ACT engine clock
                # (1-element WAR touch) so copies reusing the chunk slot
                # don't need a second sync-wait (ISA allows only one).
                nc.scalar.memzero(chunk[:, 0:1])

    _strip_redundant_self_waits(nc)
    return nc


def _strip_redundant_self_waits(nc):
    """Legalize sync waits for this ISA's small per-instruction wait slots.

    1. Engines issue and retire in order, so a wait on the instruction's
       own engine-completion semaphore whose value <= the number of prior
       same-engine instructions is guaranteed by program order. Tile emits
       such waits alongside DMA-lane waits, overflowing the single slot of
       compute/DMA encodings; strip them.
    2. The final Tile drain waits on every DMA-lane semaphore, exceeding
       the CTRL encoding's slots. A lane wait is transitively implied by
       an engine wait in the same drain when some earlier instruction on
       that engine (at tick <= the engine wait value) already waited on
       that lane at >= the required value; drop those.
    """
    from collections import defaultdict
    ticks = defaultdict(int)  # engine-completion-sem value after each inst
    # absorbed[eng][lane] = (lane_value, eng_tick_at_completion)
    absorbed = defaultdict(dict)
    for b in nc.m.functions[0].blocks:
        for inst in b.instructions:
            eng = str(inst.engine).split(".")[-1] if inst.engine else None
            si = inst.sync_info
            is_drain = type(inst).__name__ == "InstDrain"
            # this instruction's own engine-sem increment (completion tick)
            my_inc = 0
            eng_sem = None
            if si is not None and si.on_update:
                for u in si.on_update:
                    base = u.ant_name.rsplit("_", 1)[0] if u.ant_name else ""
                    if base == eng:
                        my_inc += u.update_value or 0
                        eng_sem = base
            my_tick = ticks[eng] + my_inc if eng is not None else 0
            if si is not None and si.on_wait and len(si.on_wait) > 1:
                if not is_drain:
                    keep = []
                    for w in si.on_wait:
                        base = w.ant_name.rsplit("_", 1)[0] if w.ant_name else ""
                        if (base == eng and w.wait_value is not None
                                and w.wait_value <= ticks[eng]):
                            continue  # redundant self-wait
                        keep.append(w)
                    if len(keep) < len(si.on_wait):
                        si.on_wait = keep
                else:
                    def closure(seed):
                        # transitive implications of waits in `seed`:
                        # X >= v implies Y >= val for each absorption
                        # (X waited Y >= val before X-tick <= v)
                        s = dict(seed)
                        changed = True
                        while changed:
                            changed = False
                            for x, vx in list(s.items()):
                                for y, (val, tick) in absorbed.get(x, {}).items():
                                    if tick <= (vx or 0) and val > s.get(y, -1):
                                        s[y] = val
                                        changed = True
                        return s

                    waits = {(w.ant_name.rsplit("_", 1)[0]
                              if w.ant_name else ""): w for w in si.on_wait}
                    # greedily drop waits implied by the closure of the rest
                    for base in sorted(waits, key=lambda b: absorbed.get(b) is not None):
                        others = {b: (w.wait_value or 0)
                                  for b, w in waits.items() if b != base}
                        cl = closure(others)
                        if cl.get(base, -1) >= (waits[base].wait_value or 0):
                            del waits[base]
                    keep = list(waits.values())
                    if len(keep) < len(si.on_wait):
                        si.on_wait = keep
            # record lane absorptions by this engine instruction; valid
            # evidence only if this instruction ticks its engine sem
            if (si is not None and si.on_wait and eng is not None
                    and not is_drain and my_inc > 0):
                for w in si.on_wait:
                    base = w.ant_name.rsplit("_", 1)[0] if w.ant_name else ""
                    if base and base != eng:
                        prev = absorbed[eng].get(base, (-1, -1))
                        if (w.wait_value or 0) > prev[0]:
                            absorbed[eng][base] = (w.wait_value or 0, my_tick)
            if eng is not None:
                ticks[eng] = my_tick


def _get_bass():
    if "nc" not in _BASS_CACHE:
        _BASS_CACHE["nc"] = build_bass()
    return _BASS_CACHE["nc"]


# ---------------------------------------------------------------------------
# Entry point
# ---------------------------------------------------------------------------

def run(in1, in2, trace=False):
    from concourse.bass_utils import run_bass_kernel_spmd

    if trace:
        try:
            from antenv.axon_hooks import get_axon_ntff_profile_hook  # noqa: F401
        except ImportError:
            trace = False  # NTFF hook unavailable in this container

    in1 = np.ascontiguousarray(np.asarray(in1, dtype=np.float32))
    in2 = np.ascontiguousarray(np.asarray(in2, dtype=np.float32))
    assert in1.shape == (BATCH, DIM) and in2.shape == (BATCH, DIM)

    nc = _get_bass()
    in_maps = [
        {
            "in1": np.ascontiguousarray(in1[c * ROWS_PER_CORE:(c + 1) * ROWS_PER_CORE]),
            "in2": np.ascontiguousarray(in2[c * ROWS_PER_CORE:(c + 1) * ROWS_PER_CORE]),
        }
        for c in range(N_CORES)
    ]
    res = run_bass_kernel_spmd(
        nc, in_maps, core_ids=list(range(N_CORES)), trace=trace)
    out = np.concatenate([res.results[c]["out"] for c in range(N_CORES)], axis=0)
    return out, res


def kernel(in1, in2, cb_palette=None, in1_indices=None, in2_indices=None,
           out_indices=None, cb_indices=None, cb_height=None, **_unused):
    out, _ = run(in1, in2, trace=False)
    return out
